# revision 21
# baseline (speedup 1.0000x reference)
"""Distance-aware comb-pilot interpolator for Trainium2 (8 NeuronCores).

Math: out[b, i, c] = (w_l[i] * H[b, j0(i), c] + w_r[i] * H[b, j1(i), c]) / w[i]
with pilots on the comb loc[k] = 8k (k = 0..511), Nfft = 4096.
For i = 8k + r (k < 511): j0 = k, j1 = k + 1 and the normalized weights
depend only on r:  alpha[r] = w_l/w, gamma[r] = w_r/w.
For the last 8 subcarriers (i = 4088..4095) the reference extrapolates a
virtual pilot hN = (15/8)H[511] - (7/8)H[510] at subcarrier 4095; folding it
in gives per-r coefficients on H[510] and H[511] directly.

All coefficients depend only on decay = softplus(decay_param) and are O(8)
host work; they ship to the device as one tiny [128, 48] constant tile.

Device kernel (per core, batch-sharded 512 rows): partition dim = batch.
Per 128-batch tile: ScalarE computes tmp = gamma[r] * H[:, k+1, :], then the
DVE fused op scalar_tensor_tensor writes out[:, k, r, :] =
(H[:, k, :] * alpha[r]) + tmp for all k in one pass.  The kernel moves
2.1 MB in / 16.8 MB out per core and is HBM-bound (~53 us roofline).
"""

import sys

import numpy as np

for _p in ("/opt/trn_rl_repo", "/root/.axon_site/_ro/trn_rl_repo"):
    if _p not in sys.path:
        sys.path.append(_p)

import concourse.bass as bass
import concourse.tile as tile
from concourse import bacc, mybir
from concourse.bass_utils import run_bass_kernel_spmd

N_CORES = 8
B, NP, NFFT, SPACING = 4096, 512, 4096, 8
B_LOC = B // N_CORES  # batch rows per core
NSEG = NP - 1  # regular 8-wide segments (k = 0..510)
P = 128  # SBUF partitions
N_BT = B_LOC // P  # 128-batch tiles per core

_PROGRAM = None


def _build_program():
    """One Bass program, identical on all cores (pure data parallel)."""
    nc = bacc.Bacc("TRN2", target_bir_lowering=False, debug=False)
    f32 = mybir.dt.float32
    ls = nc.dram_tensor("ls", [B_LOC, NP * 2], f32, kind="ExternalInput").ap()
    coef = nc.dram_tensor("coef", [P, 64], f32, kind="ExternalInput").ap()
    out = nc.dram_tensor("out", [B_LOC, NFFT * 2], f32, kind="ExternalOutput").ap()

    mult, add = mybir.AluOpType.mult, mybir.AluOpType.add

    # Output k-chunking per 128-batch tile (DVE op + output-DMA granularity)
    # and ScalarE mul ranges (tmp granularity), decoupled: per-op fixed cost
    # is high (ACT ~0.39us, DVE ~0.2us), so both engines run big ops except
    # where chunk timing matters — a small leading chunk on tile 0 starts
    # the output stream early, and a small trailing chunk on tile 3 keeps
    # the post-compute DMA drain short.
    CHUNKS = {
        0: [(0, 128), (128, NSEG)],
        1: [(0, NSEG)],
        2: [(0, NSEG)],
        3: [(0, 384), (384, 448), (448, NSEG)],
    }
    MULS = {
        0: [(128, NSEG)],
        1: [(0, NSEG)],
        2: [(0, NSEG)],
        3: [(0, NSEG)],
    }
    # tile 0 chunk (0,128) uses the factored form out = gamma*(rho*H + Hn):
    # the DVE op needs no ScalarE tmp, so the first output chunk (and the
    # whole store stream) starts ~1.5 us earlier.
    FACTORED = {(0, 0)}

    with tile.TileContext(nc) as tc:
        with (
            tc.tile_pool(name="cpool", bufs=1) as cpool,
            tc.tile_pool(name="hpool", bufs=4) as hpool,
            tc.tile_pool(name="opool", bufs=3) as opool,
            tc.tile_pool(name="tpool", bufs=12) as tpool,
            tc.tile_pool(name="lpool", bufs=2) as lpool,
        ):
            # Preload every input before any output traffic exists — loads
            # issued mid-kernel crawl behind the output bursts (SDMA packet
            # round-robin across queues). The first compute op needs coef +
            # the first 258 columns of h0, so those two small DMAs go first
            # on HWDGE (sync); everything else on SWDGE (gpsimd) to stay off
            # the store queue. h0 is split in two tiles (h0a covers k<129,
            # h0b covers k>=128 with a 2-column overlap) so the first
            # ScalarE op isn't gated on the full 512 KB h0 transfer.
            # ct/h0a trigger from ScalarE's HWDGE (qActDynamicHW): ScalarE's
            # preamble ends ~1.5 us before Sync's first possible trigger, and
            # this ring is separate from the store ring entirely.
            H0A = 258  # columns of h0a = pilots k <= 128 (incl c pair)
            ct = cpool.tile([P, 64], f32)
            nc.scalar.dma_start(ct[:], coef)
            h0a = hpool.tile([P, H0A], f32, name="h0a", tag="h0a")
            nc.scalar.dma_start(h0a[:], ls[0:P, 0:H0A])
            h0b = hpool.tile([P, NP * 2 - 256], f32, name="h0b", tag="h0b")
            nc.gpsimd.dma_start(h0b[:], ls[0:P, 256:])
            hs = [
                (h0a, h0b) if t == 0
                else hpool.tile([P, NP * 2], f32, name="h", tag="h")
                for t in range(N_BT)
            ]
            for t in range(1, N_BT):
                nc.gpsimd.dma_start(hs[t][:], ls[t * P : (t + 1) * P, :])

            def hcols(t, c0, c1):
                """AP over h columns [c0, c1) of tile t (handles split h0)."""
                if t == 0:
                    h0a, h0b = hs[0]
                    if c1 <= H0A:
                        return h0a[:, c0:c1]
                    assert c0 >= 256, (c0, c1)
                    return h0b[:, c0 - 256 : c1 - 256]
                return hs[t][:, c0:c1]

            def hseg(t, k0, k1):
                """[P, k1-k0, 2] view of pilots k0..k1 of tile t."""
                return hcols(t, 2 * k0, 2 * k1).rearrange("p (k c) -> p k c", c=2)

            for t in range(N_BT):
                o = opool.tile([P, NFFT * 2], f32)
                ov = o[:].rearrange("p (k r c) -> p k r c", r=SPACING, c=2)

                # tmp[r] = gamma[r] * H[:, k+1, :] over MULS ranges (ScalarE)
                tmps = {}
                for m0, m1 in MULS[t]:
                    for r in range(SPACING):
                        tmp = tpool.tile([P, NSEG, 2], f32, name="tmp", tag="tmp")
                        nc.scalar.mul(
                            tmp[:, 0 : m1 - m0, :],
                            hseg(t, m0 + 1, m1 + 1),
                            ct[:, 8 + r : 9 + r],
                        )
                        tmps[(m0, r)] = tmp

                for ci, (k0, k1) in enumerate(CHUNKS[t]):
                    last = ci == len(CHUNKS[t]) - 1
                    for r in range(SPACING):
                        if (t, k0) in FACTORED:
                            # u = rho[r]*H + Hn (DVE, no tmp dep), out = gamma[r]*u
                            u = tpool.tile([P, 128, 2], f32, name="u", tag="u", bufs=4)
                            nc.vector.scalar_tensor_tensor(
                                u[:, 0 : k1 - k0, :],
                                hseg(t, k0, k1),
                                ct[:, 48 + r : 49 + r],
                                hseg(t, k0 + 1, k1 + 1),
                                mult,
                                add,
                            )
                            nc.scalar.mul(
                                ov[:, k0:k1, r, :],
                                u[:, 0 : k1 - k0, :],
                                ct[:, 8 + r : 9 + r],
                            )
                            continue
                        # find the mul range containing [k0, k1)
                        m0, m1 = next(m for m in MULS[t] if m[0] <= k0 and k1 <= m[1])
                        tv = tmps[(m0, r)][:, k0 - m0 : k1 - m0, :]
                        # out[:, k, r, :] = alpha[r]*H[:, k, :] + tmp  (fused DVE)
                        nc.vector.scalar_tensor_tensor(
                            ov[:, k0:k1, r, :],
                            hseg(t, k0, k1),
                            ct[:, r : r + 1],
                            tv,
                            mult,
                            add,
                        )

                    if last:
                        # Last 8 subcarriers: coeffs vary along r — broadcast
                        # H[510]/H[511] against per-element coef tiles, on the
                        # otherwise-idle GpSimd engine (off the critical path:
                        # these columns are independent of the DVE ops).
                        h510 = hcols(t, 2 * NP - 4, 2 * NP - 2).unsqueeze(1).broadcast_to((P, 8, 2))
                        h511 = hcols(t, 2 * NP - 2, 2 * NP).unsqueeze(1).broadcast_to((P, 8, 2))
                        a_last = ct[:, 16:32].rearrange("p (r c) -> p r c", c=2)
                        c_last = ct[:, 32:48].rearrange("p (r c) -> p r c", c=2)
                        tl = lpool.tile([P, 8, 2], f32)
                        nc.gpsimd.tensor_mul(tl[:], h510, a_last)
                        t2 = lpool.tile([P, 8, 2], f32)
                        nc.gpsimd.tensor_mul(t2[:], h511, c_last)
                        o_last = o[:, NSEG * 16 : NFFT * 2].rearrange("p (r c) -> p r c", c=2)
                        nc.gpsimd.tensor_add(o_last, tl[:], t2[:])

                    lo = k0 * 16
                    hi = NFFT * 2 if last else k1 * 16
                    nc.sync.dma_start(
                        out[t * P : (t + 1) * P, lo:hi], o[:, lo:hi]
                    )
    nc.compile()
    return nc


def _coef_tile(decay_param: np.ndarray) -> np.ndarray:
    """[128, 48] f32: cols 0:8 alpha[r], 8:16 gamma[r], 16:32 last-chunk
    coeff on H[510] (r,c-flattened), 32:48 last-chunk coeff on H[511]."""
    x = np.float32(np.asarray(decay_param).reshape(-1)[0])
    d = np.logaddexp(np.float32(0.0), x, dtype=np.float32)  # softplus
    r = np.arange(SPACING, dtype=np.float32)
    eps = np.float32(1e-12)
    # regular segments: x1 - x0 = 8
    wl = np.exp(-d * r, dtype=np.float32)
    wr = np.exp(-d * (np.float32(SPACING) - r), dtype=np.float32)
    w = wl + wr + eps
    alpha, gamma = wl / w, wr / w
    # last chunk: i = 4088 + r, x0 = 4088, x1 = 4095 (gap of 7);
    # y1 = hN = (15/8) H[511] - (7/8) H[510]
    wl2 = np.exp(-d * r, dtype=np.float32)
    wr2 = np.exp(-d * (np.float32(7.0) - r), dtype=np.float32)
    w2 = wl2 + wr2 + eps
    c511 = (wl2 + np.float32(1.875) * wr2) / w2
    c510 = -np.float32(0.875) * wr2 / w2
    # rho = alpha/gamma = exp(d*(8-2r)) for the factored first chunk
    # (out = gamma*(rho*H + Hn)); guards only matter for absurd decay.
    rho = np.clip(alpha / np.maximum(gamma, np.float32(1e-30)), 0, 3.0e38).astype(
        np.float32
    )
    row = np.concatenate(
        [alpha, gamma, np.repeat(c510, 2), np.repeat(c511, 2),
         rho, np.zeros(8, np.float32)]
    ).astype(np.float32)
    return np.broadcast_to(row, (P, 64)).copy()


def kernel(LS_ri, pilot_pos=None, decay_param=None, Nfft=None, **_unused):
    global _PROGRAM
    LS_ri = np.ascontiguousarray(np.asarray(LS_ri, dtype=np.float32))
    coef = _coef_tile(decay_param)

    if _PROGRAM is None:
        _PROGRAM = _build_program()
    nc = _PROGRAM

    in_maps = []
    for c in range(N_CORES):
        shard = LS_ri[c * B_LOC : (c + 1) * B_LOC].reshape(B_LOC, NP * 2)
        in_maps.append({"ls": shard, "coef": coef})

    res = run_bass_kernel_spmd(nc, in_maps, list(range(N_CORES))).results
    out = np.concatenate(
        [res[c]["out"].reshape(B_LOC, NFFT, 2) for c in range(N_CORES)], axis=0
    )
    return out
